# revision 5
# baseline (speedup 1.0000x reference)
"""Trainium2 Bass kernel for nn_MAB_65068754534455 (dense transformer MAB block).

Computation (per reference):
  q = query @ Wq.T + bq ; k = kv @ Wk.T + bk ; v = kv @ Wv.T + bv
  per head: A = softmax(q k^T / sqrt(hd)) ; o = A v
  x = qheads + o (merged) ; out = x + relu(x @ Wo.T + bo)

Sharding: 8 cores = 4 batches x 2 query-halves (data parallel, no collectives).

v2 design (from trace analysis of the v1 kernel):
 - bf16 inputs/weights (halves input DMA bytes), fine-grained DMA chunks and
   interleaved projection emission so the exp stream starts ~10us in instead
   of ~85us.
 - softmax exp is the critical resource (16.8M elements/core, ScalarE-only at
   1 elem/cycle/lane): ~1/4 of score tiles are exp'd on the Vector engine via
   two custom DVE ops (degree-4 polynomial for exp(x/16), then x^16), fitted
   to the same accuracy as the bf16 e^x storage itself.
 - scores matmuls for the head pair run concurrently via PE row tiling
   (K=64 at tile_position (0,0) and (64,0)).
 - softmax normalize: o copied PSUM->SBUF once (releases the PSUM bank for
   the next slot), reciprocal via the fast approx DVE op, broadcasts and
   residual adds on GpSimd.
 - PSUM budget: scores 2x2 banks + o 1x2 banks + proj/z 2x1 banks = 8.
 - loop order (qb outer) so the fc_o stage for the first query half overlaps
   attention of the second half; outputs stream per (j, qb) chunk.
"""

import math

import ml_dtypes
import numpy as np

import concourse.mybir as mybir
import concourse.tile as tile
from concourse import bacc
from concourse.bass_utils import run_bass_kernel_spmd

# ---------------------------------------------------------------- constants
B, SQ, SKV, D, H = 4, 2048, 2048, 512, 8
HD = D // H                      # 64
SCALE = 1.0 / math.sqrt(HD)
NCORES = 8
TQ = SQ // 2                     # 1024 query rows per core

F32 = mybir.dt.float32
BF16 = mybir.dt.bfloat16

KT = D // 128                    # 4 contraction k-tiles
DT = D // 128                    # 4 output d-tiles
NQB = TQ // 512                  # 2 query blocks of 512
NKB = SKV // 512                 # 4 kv blocks of 512
NTK = SKV // 128                 # 16 kv tiles of 128
VW = HD + 1                      # 65: V head block width incl. ones column

# score tiles per slot handled by the DVE exp path (rest on ScalarE)
DVE_EXP_I = (2, 5, 8, 11)

# ------------------------------------------------- custom DVE exp operators
# exp(s*SCALE) ~= p(s)^16,  p = deg-4 minimax poly of exp(u) on |u|<=0.55,
# u = s*SCALE/16.  End-to-end softmax error identical to bf16 storage of
# exact exp (~1.5e-3 on o), far inside the 2e-2 gate.
_PA, _PB, _PC, _PD = 0.99953732, 0.4999736, 0.17062412, 0.04212414
_KU = SCALE / 16.0
EXP_C0 = float(_PD * _KU**4)     # s0   (u^4 coef)
EXP_C1 = float(_PC * _KU**3)     # s1   (u^3)
EXP_C2 = float(_PB * _KU**2)     # imm2 (u^2)
EXP_C3 = float(_PA * _KU)        # via in1 [P,1] (u^1; C3 spills to Src1)


def _register_exp_ops():
    import concourse.dve_ops as dvo
    from concourse.dve_spec import (
        C0, C1, C2, C3, One, Spec, Src0, _has_src1, _spill_c3_to_src1, lower, sq,
    )
    from concourse.dve_table_gen import dve_ver_for
    from concourse.dve_uop import DveOpSpec

    def mk(name, spec):
        for op in dvo.OPS:       # idempotent under re-import
            if op.name == name:
                return op
        ver = dve_ver_for("TRN2")
        row = dvo._CUSTOM_DVE_ROW_BASE + len(dvo.OPS)
        assert row < 0x20
        tmp = DveOpSpec(
            name=name, opcode=row, uops=lower(spec, ver=ver),
            rd1_en=_has_src1(spec),
        )
        op = dvo.DveOp(name, spec, subdim=False, uops_sha={ver: tmp.sha(ver)})
        dvo.OPS.append(op)
        dvo._SUB_OPCODE_FOR_NAME[name] = row
        dvo.CUSTOM_DVE_SPECS[name] = spec
        return op

    def ref1(in0, in1, s0, s1, imm2):
        x = in0.astype(np.float32)
        return ((((s0 * x + s1) * x + imm2) * x + in1) * x + 1.0).astype(
            np.float32
        )

    body1 = _spill_c3_to_src1(
        (((C0 * Src0 + C1) * Src0 + C2) * Src0 + C3) * Src0 + One
    )
    op1 = mk("EXP_POLY4_ANT", Spec(body=body1, reference=ref1))

    def ref2(in0, in1, s0, s1, imm2):
        x = in0.astype(np.float32)
        x = x * x
        x = x * x
        x = x * x
        return (x * x).astype(np.float32)

    op2 = mk("POW16_ANT", Spec(body=sq(sq(sq(sq(Src0)))), reference=ref2))
    return op1, op2


EXP_P1, EXP_P2 = _register_exp_ops()


# ------------------------------------------------------------------- kernel
def _build():
    nc = bacc.Bacc(None, target_bir_lowering=False, debug=False)

    xqt = nc.dram_tensor("xqt", [D, TQ], BF16, kind="ExternalInput").ap()
    xkvt = nc.dram_tensor("xkvt", [D, SKV], BF16, kind="ExternalInput").ap()
    wqt = nc.dram_tensor("wqt", [D, D], BF16, kind="ExternalInput").ap()
    wkt = nc.dram_tensor("wkt", [D, D], BF16, kind="ExternalInput").ap()
    wvt = nc.dram_tensor("wvt", [D, D], BF16, kind="ExternalInput").ap()
    wot = nc.dram_tensor("wot", [D, D], BF16, kind="ExternalInput").ap()
    bq4 = nc.dram_tensor("bq4", [128, DT], F32, kind="ExternalInput").ap()
    bk4 = nc.dram_tensor("bk4", [128, DT], F32, kind="ExternalInput").ap()
    bo4 = nc.dram_tensor("bo4", [128, DT], F32, kind="ExternalInput").ap()
    bvb = nc.dram_tensor("bvb", [128, D], F32, kind="ExternalInput").ap()
    outt = nc.dram_tensor("outt", [D, TQ], F32, kind="ExternalOutput").ap()

    with tile.TileContext(nc) as tc:
        with (
            tc.tile_pool(name="persist", bufs=1) as pp,
            tc.tile_pool(name="e2", bufs=4) as ep,
            tc.tile_pool(name="pexp", bufs=2) as pxp,
            tc.tile_pool(name="oc", bufs=2) as ocp,
            tc.tile_pool(name="rbi", bufs=2) as rip,
            tc.tile_pool(name="rb", bufs=2) as rbp,
            tc.tile_pool(name="on", bufs=2) as onp,
            tc.tile_pool(name="on64", bufs=2) as o64p,
            tc.tile_pool(name="yt", bufs=3) as yp,
            tc.tile_pool(name="s2", bufs=2, space="PSUM") as sp,
            tc.tile_pool(name="ops", bufs=1, space="PSUM") as opl,
            tc.tile_pool(name="pj", bufs=2, space="PSUM") as pjp,
        ):
            w_q = pp.tile([128, KT, D], BF16)
            w_k = pp.tile([128, KT, D], BF16)
            w_v = pp.tile([128, KT, D], BF16)
            w_o = pp.tile([128, KT, D], BF16)
            xq_s = pp.tile([128, KT, TQ], BF16)
            xkv_s = pp.tile([128, KT, SKV], BF16)
            qtb = pp.tile([128, DT, TQ], BF16)     # q^T, becomes x^T
            kt = pp.tile([128, DT, SKV], BF16)     # k^T (scores lhsT)
            v = pp.tile([128, NTK, H * VW], BF16)  # V with ones cols (PV lhsT)
            bq_s = pp.tile([128, DT], F32)
            bk_s = pp.tile([128, DT], F32)
            bo_s = pp.tile([128, DT], F32)
            bv_s = pp.tile([128, D], F32)
            c3t = pp.tile([128, 1], F32)

            # ---- input DMA, in consumption order, chunked ----
            xkv_r = xkvt.rearrange("(o p) t -> p o t", p=128)
            xq_r = xqt.rearrange("(o p) t -> p o t", p=128)
            nc.sync.dma_start(bk_s[:], bk4[:])
            nc.sync.dma_start(bq_s[:], bq4[:])
            nc.sync.dma_start(w_k[:], wkt.rearrange("(o p) d -> p o d", p=128))
            nc.sync.dma_start(xkv_s[:, :, 0:512], xkv_r[:, :, 0:512])
            nc.sync.dma_start(w_q[:], wqt.rearrange("(o p) d -> p o d", p=128))
            nc.sync.dma_start(xq_s[:, :, 0:512], xq_r[:, :, 0:512])
            for b in range(1, NKB):
                bsl = slice(b * 512, (b + 1) * 512)
                nc.sync.dma_start(xkv_s[:, :, bsl], xkv_r[:, :, bsl])
            nc.sync.dma_start(bv_s[:], bvb[:])
            nc.sync.dma_start(w_v[:], wvt.rearrange("(o p) d -> p o d", p=128))
            nc.sync.dma_start(xq_s[:, :, 512:1024], xq_r[:, :, 512:1024])
            nc.sync.dma_start(bo_s[:], bo4[:])
            nc.sync.dma_start(w_o[:], wot.rearrange("(o p) d -> p o d", p=128))

            nc.gpsimd.memset(c3t[:], EXP_C3)
            # ones columns of V (col 64 of each 65-wide head block)
            for i in range(NTK):
                nc.gpsimd.memset(
                    v[:, i, :].rearrange("p (h w) -> p h w", w=VW)[:, :, HD], 1.0
                )

            # ---- projection emitters ----
            def kproj(j, b):
                bsl = slice(b * 512, (b + 1) * 512)
                ps = pjp.tile([128, 512], F32, tag="pj", name="pjt")
                for k in range(KT):
                    nc.tensor.matmul(
                        ps[:], w_k[:, k, j * 128 : (j + 1) * 128],
                        xkv_s[:, k, bsl], start=(k == 0), stop=(k == KT - 1),
                    )
                nc.vector.tensor_scalar_add(
                    kt[:, j, bsl], ps[:], bk_s[:, j : j + 1]
                )

            def qproj(j, qb):
                qsl = slice(qb * 512, (qb + 1) * 512)
                ps = pjp.tile([128, 512], F32, tag="pj", name="pjt")
                for k in range(KT):
                    nc.tensor.matmul(
                        ps[:], w_q[:, k, j * 128 : (j + 1) * 128],
                        xq_s[:, k, qsl], start=(k == 0), stop=(k == KT - 1),
                    )
                nc.vector.tensor_scalar_add(
                    qtb[:, j, qsl], ps[:], bq_s[:, j : j + 1]
                )

            def vproj(i):
                ps = pjp.tile([128, 512], F32, tag="pj", name="pjt")
                for k in range(KT):
                    nc.tensor.matmul(
                        ps[:], xkv_s[:, k, i * 128 : (i + 1) * 128], w_v[:, k, :],
                        start=(k == 0), stop=(k == KT - 1),
                    )
                nc.vector.tensor_tensor(
                    v[:, i, :].rearrange("p (h w) -> p h w", w=VW)[:, :, 0:HD],
                    ps[:].rearrange("p (h w) -> p h w", w=HD),
                    bv_s[:].rearrange("p (h w) -> p h w", w=HD),
                    mybir.AluOpType.add,
                )

            def phase3(j, qb):
                qsl = slice(qb * 512, (qb + 1) * 512)
                z = pjp.tile([128, 512], F32, tag="pj", name="pjt")
                for k in range(KT):
                    nc.tensor.matmul(
                        z[:], w_o[:, k, j * 128 : (j + 1) * 128],
                        qtb[:, k, qsl], start=(k == 0), stop=(k == KT - 1),
                    )
                yt = yp.tile([128, 512], F32)
                nc.vector.tensor_scalar(
                    yt[:], z[:], bo_s[:, j : j + 1], 0.0,
                    mybir.AluOpType.add, mybir.AluOpType.max,
                )
                nc.gpsimd.tensor_add(yt[:], yt[:], qtb[:, j, qsl])
                nc.sync.dma_start(outt[j * 128 : (j + 1) * 128, qsl], yt[:])

            # ---- pre-phase: minimum work before the first attention slot ----
            kproj(0, 0)
            qproj(0, 0)
            kproj(0, 1)
            kproj(0, 2)
            kproj(0, 3)
            for i in range(8):
                vproj(i)

            # ---- extra emissions interleaved into each slot, keyed by
            #      (slot index, i position) ----
            extras = {}

            def sched(s, i, fn, *args):
                extras.setdefault((s, i), []).append((fn, args))

            for n, i0 in enumerate(range(8, NTK)):      # V tiles 8..15 in s0
                sched(0, 1 + n, vproj, i0)
            for b in range(NKB):                        # K j=1 in s0
                sched(0, 9 + b, kproj, 1, b)
            sched(0, 14, qproj, 1, 0)
            for b in range(NKB):                        # K j=2 in s1
                sched(1, 2 + 3 * b, kproj, 2, b)
            sched(1, 13, qproj, 2, 0)
            for b in range(NKB):                        # K j=3 in s2
                sched(2, 2 + 3 * b, kproj, 3, b)
            sched(2, 13, qproj, 3, 0)
            sched(3, 2, qproj, 0, 1)                    # Q for qb1 in s3
            sched(3, 8, qproj, 1, 1)
            sched(4, 2, qproj, 2, 1)
            sched(4, 8, qproj, 3, 1)
            sched(5, 2, phase3, 0, 0)                   # fc_o for qb0
            sched(5, 9, phase3, 1, 0)
            sched(6, 2, phase3, 2, 0)
            sched(6, 9, phase3, 3, 0)

            # ---- attention slots ----
            ACT_EXP = mybir.ActivationFunctionType.Exp
            s = 0
            for qb in range(NQB):
                qsl = slice(qb * 512, (qb + 1) * 512)
                for hp in range(H // 2):
                    o = opl.tile([65, 2, 512], F32, tag="o", name="ot")
                    h_e, h_o = 2 * hp, 2 * hp + 1
                    for i in range(NTK):
                        for fn, args in extras.get((s, i), ()):
                            fn(*args)
                        isl = slice(i * 128, (i + 1) * 128)
                        s2 = sp.tile([128, 2, 512], F32, tag="s2", name="s2t")
                        nc.tensor.matmul(
                            s2[:, 0, :], kt[0:64, hp, isl], qtb[0:64, hp, qsl],
                            start=True, stop=True, tile_position=(0, 0),
                        )
                        nc.tensor.matmul(
                            s2[:, 1, :], kt[64:128, hp, isl],
                            qtb[64:128, hp, qsl],
                            start=True, stop=True, tile_position=(64, 0),
                        )
                        e2 = ep.tile([128, 2, 512], BF16)
                        if i in DVE_EXP_I:
                            p = pxp.tile([128, 2, 512], F32)
                            nc.vector._custom_dve(
                                EXP_P1, out=p[:], in0=s2[:], in1=c3t[:],
                                s0=EXP_C0, s1=EXP_C1, imm2=EXP_C2,
                            )
                            nc.vector._custom_dve(EXP_P2, out=e2[:], in0=p[:])
                        else:
                            nc.scalar.activation(
                                e2[:], s2[:], ACT_EXP, scale=SCALE
                            )
                        nc.tensor.matmul(
                            o[:, 0, :], v[:, i, h_e * VW : (h_e + 1) * VW],
                            e2[:, 0, :], start=(i == 0), stop=(i == NTK - 1),
                        )
                        nc.tensor.matmul(
                            o[:, 1, :], v[:, i, h_o * VW : (h_o + 1) * VW],
                            e2[:, 1, :], start=(i == 0), stop=(i == NTK - 1),
                        )
                    # normalize + residual; copy releases the o PSUM banks
                    oc = ocp.tile([65, 2, 512], F32)
                    nc.vector.tensor_copy(oc[:], o[:])
                    rr0 = rip.tile([1, 2, 512], F32)
                    nc.sync.dma_start(rr0[:], oc[64:65, :, :])
                    rbi = rip.tile([1, 2, 512], F32)
                    nc.vector.reciprocal_approx_fast(out=rbi[:], in_=rr0[:])
                    rb = rbp.tile([64, 2, 512], F32)
                    nc.gpsimd.partition_broadcast(rb[:, 0, :], rbi[0:1, 0, :])
                    nc.gpsimd.partition_broadcast(rb[:, 1, :], rbi[0:1, 1, :])
                    on = onp.tile([64, 2, 512], BF16)
                    nc.vector.tensor_mul(on[:], oc[0:64, :, :], rb[:])
                    nc.vector.tensor_add(
                        qtb[0:64, hp, qsl], qtb[0:64, hp, qsl], on[:, 0, :]
                    )
                    on64 = o64p.tile([128, 512], BF16)
                    nc.sync.dma_start(on64[64:128, :], on[:, 1, :])
                    nc.vector.tensor_add(
                        qtb[64:128, hp, qsl], qtb[64:128, hp, qsl],
                        on64[64:128, :],
                    )
                    s += 1

            # ---- tail: fc_o for qb1 ----
            for j in range(DT):
                phase3(j, 1)

    nc.compile()
    return nc


_NC = None


def _get_nc():
    global _NC
    if _NC is None:
        _NC = _build()
    return _NC


def kernel(**inputs) -> np.ndarray:
    bf = ml_dtypes.bfloat16
    A = np.ascontiguousarray
    q = np.asarray(inputs["query"], dtype=np.float32)
    kv = np.asarray(inputs["key_value"], dtype=np.float32)
    shared = {
        "wqt": A(np.asarray(inputs["Wq"], np.float32).T.astype(bf)),
        "wkt": A(np.asarray(inputs["Wk"], np.float32).T.astype(bf)),
        "wvt": A(np.asarray(inputs["Wv"], np.float32).T.astype(bf)),
        "wot": A(np.asarray(inputs["Wo"], np.float32).T.astype(bf)),
        "bq4": A(np.asarray(inputs["bq"], np.float32).reshape(DT, 128).T),
        "bk4": A(np.asarray(inputs["bk"], np.float32).reshape(DT, 128).T),
        "bo4": A(np.asarray(inputs["bo"], np.float32).reshape(DT, 128).T),
        "bvb": A(np.broadcast_to(np.asarray(inputs["bv"], np.float32), (128, D))),
    }
    in_maps = []
    for c in range(NCORES):
        b, half = divmod(c, 2)
        qs = q[b, half * TQ : (half + 1) * TQ]
        in_maps.append(
            {
                "xqt": A(qs.T.astype(bf)),
                "xkvt": A(kv[b].T.astype(bf)),
                **shared,
            }
        )

    nc = _get_nc()
    res = run_bass_kernel_spmd(nc, in_maps, core_ids=list(range(NCORES)))
    kernel._last_results = res  # for test harness introspection

    out = np.empty((B, SQ, D), np.float32)
    for c in range(NCORES):
        b, half = divmod(c, 2)
        out[b, half * TQ : (half + 1) * TQ] = res.results[c]["outt"].T
    return out
